# revision 43
# baseline (speedup 1.0000x reference)
"""GATv2 message passing on 8 Trainium2 NeuronCores (Bass/Tile), v2.

Strategy (edge-parallel by receiver ownership, fp16 pipeline):
  - Host permutes nodes -> (core, window, slot) balancing per-window edge
    counts (lo/hi sender halves) so the SPMD-uniform chunks-per-window is
    near the average instead of the max (pad ~7% vs ~25%).
  - Each core projects the full node table with Ws into fp16 DRAM tables
    (lo/hi split for int16 gather indices) and its local permuted slice
    with Wr into an SBUF-resident fp16 table r_sb[slot, window, feat].
  - Phase 2 streams one window per iteration: SWDGE-gathers the fp16
    s-projection rows (the only gpsimd work), reconstructs per-edge
    r-rows with a one-hot matmul ohT^T @ r_win on the PE (no r gather),
    computes mish + attention logits in fp16 (act engine: Exp, Square,
    Copy-affine; DVE: recip + muls), and scatter-adds exp(logit)*msg and
    exp(logit) into a single 136-column PSUM accumulator via one one-hot
    matmul per chunk.
  - exp uses bias attn_b - 2 (cancels in the softmax ratio, keeps fp16
    products in range).  out[n] = agg/den computed on device in f32.
"""

import os
import sys

for _p in ("/opt/trn_rl_repo", "/root/.axon_site/_ro/trn_rl_repo"):
    if os.path.isdir(_p) and _p not in sys.path:
        sys.path.insert(0, _p)

import numpy as np

import concourse.bass as bass
import concourse.bacc as bacc
import concourse.tile as tile
from concourse import mybir
from concourse import bass_utils

F32 = mybir.dt.float32
F16 = mybir.dt.float16
I16 = mybir.dt.int16
I32 = mybir.dt.int32

N_NODES = 50000
N_EDGES = 800000
F = 128            # feature dim
H = 8              # heads
D = 16             # head dim
NCORE = 8
NPC = N_NODES // NCORE          # 6250 nodes per core
WIN = 128                       # nodes (slots) per scatter window
NWIN = 49                       # windows per core (49*128 = 6272 slots)
NSLOT = NWIN * WIN              # 6272
SPLIT = 32768                   # int16 gather-index limit -> lo/hi tables
NP_PAD = 50176                  # global nodes padded to 98*512
HI_ROWS = NP_PAD - SPLIT        # 17408
CHUNK = 128                     # edges per matmul chunk
RIFF = 4                        # chunks per PSUM riff (2KB bank)
GCAP = 8                        # max chunks per dma_gather call

_prog_cache = {}


def _build_program(L_ch, H_ch, exp_bias):
    """SPMD Bass program: L_ch lo chunks + H_ch hi chunks per window."""
    cpw = L_ch + H_ch
    wine = cpw * CHUNK                       # edges per window (padded)

    nc = bacc.Bacc("TRN2", target_bir_lowering=False, debug=False,
                   enable_asserts=False, num_devices=NCORE)

    def dram_in(name, shape, dt=F16):
        return nc.dram_tensor(name, list(shape), dt, kind="ExternalInput").ap()

    nodes_T = dram_in("nodes_T", (F, NP_PAD))
    nloc_T = dram_in("nloc_T", (F, NSLOT))
    ws_mat = dram_in("ws_mat", (F, F))
    wr_mat = dram_in("wr_mat", (F, F))
    wsb_rep = dram_in("wsb_rep", (128, F))   # bias row replicated to 128 parts
    wrb_rep = dram_in("wrb_rep", (128, F))
    iota_in = dram_in("iota", (128, 128))    # value = free idx
    attn_in = dram_in("attn_rep", (128, 128))
    sidx_in = dram_in("sidx", (NWIN, 128, wine // 16), I16)
    rloc_in = dram_in("rloc", (NWIN, 128, cpw))
    ohT_in = dram_in("ohT", (NWIN, 128, wine))   # [w, n, c*128+e]
    out_d = nc.dram_tensor("out_d", [NSLOT, F], F32, kind="ExternalOutput").ap()

    tab_lo = nc.dram_tensor("tab_lo", [SPLIT, F], F16, kind="Internal").ap()
    tab_hi = nc.dram_tensor("tab_hi", [HI_ROWS, F], F16, kind="Internal").ap()
    segs = []
    pos = 0
    for nch_total, tab in ((L_ch, "lo"), (H_ch, "hi")):
        left = nch_total
        while left > 0:
            n = min(GCAP, left)
            segs.append((tab, pos, n))
            pos += n
            left -= n
    riffs = [(r0, min(RIFF, cpw - r0)) for r0 in range(0, cpw, RIFF)]
    NB = 8                                   # windows per normalize batch

    with nc.allow_low_precision(reason="fp16 pipeline, tol 2e-2"), \
         tile.TileContext(nc) as tc:
        with tc.tile_pool(name="const", bufs=1) as cpool, \
             tc.tile_pool(name="stage", bufs=3) as stpool, \
             tc.tile_pool(name="gat", bufs=3) as gpool, \
             tc.tile_pool(name="work", bufs=2) as wpool, \
             tc.tile_pool(name="proj_ps", bufs=2, space="PSUM") as ppool, \
             tc.tile_pool(name="psR", bufs=4, space="PSUM") as psR, \
             tc.tile_pool(name="psA", bufs=2, space="PSUM") as psA:
            ws_t = cpool.tile([F, F], F16)
            wr_t = cpool.tile([F, F], F16)
            wsbr_t = cpool.tile([128, F], F16)
            wrbr_t = cpool.tile([128, F], F16)
            iota_t = cpool.tile([128, 128], F16)
            attn_t = cpool.tile([128, 128], F16)
            r_sb = cpool.tile([128, NWIN, F], F16)       # [slot, win, feat]
            acc = cpool.tile([128, NWIN, F + H], F32)    # [slot, win, agg|den]
            b_exp = cpool.tile([128, 1], F32)            # exp bias const
            c_m2 = cpool.tile([128, 1], F32)             # -2.0 scale const
            nc.vector.memset(b_exp[:], float(exp_bias))
            nc.vector.memset(c_m2[:], -2.0)
            nc.sync.dma_start(out=ws_t[:], in_=ws_mat[:])
            nc.sync.dma_start(out=wr_t[:], in_=wr_mat[:])
            nc.sync.dma_start(out=wsbr_t[:], in_=wsb_rep[:])
            nc.sync.dma_start(out=wrbr_t[:], in_=wrb_rep[:])
            nc.sync.dma_start(out=iota_t[:], in_=iota_in[:])
            nc.sync.dma_start(out=attn_t[:], in_=attn_in[:])

            # ------------- s-table projection ------------------------------
            for g in range(NP_PAD // 512):
                xT = stpool.tile([128, RIFF, 128], F16, tag="pp_x")
                nc.sync.dma_start(
                    out=xT[:],
                    in_=nodes_T[:, g * 512:(g + 1) * 512]
                        .rearrange("p (c n) -> p c n", n=128))
                ps = ppool.tile([128, RIFF, 128], F32, space="PSUM",
                                tag="proj")
                for c in range(RIFF):
                    nc.tensor.matmul(ps[:, c, :], lhsT=xT[:, c, :],
                                     rhs=ws_t[:], start=True, stop=True,
                                     skip_group_check=True)
                y = stpool.tile([128, RIFF, 128], F16, tag="pp_y")
                nc.vector.tensor_tensor(
                    y[:], ps[:],
                    wsbr_t[:].unsqueeze(1).to_broadcast([128, RIFF, 128]),
                    op=mybir.AluOpType.add)
                row = g * 512
                if row < SPLIT:
                    dst = tab_lo[row:row + 512, :]
                else:
                    dst = tab_hi[row - SPLIT:row - SPLIT + 512, :]
                nc.sync.dma_start(
                    out=dst.rearrange("(c p) f -> p c f", p=128),
                    in_=y[:])

            tc.strict_bb_all_engine_barrier()

            # ------------- r projection straight into SBUF ------------------
            w0 = 0
            while w0 < NWIN:
                nch = min(4, NWIN - w0)
                xT = stpool.tile([128, RIFF, 128], F16, tag="pp_x")
                nc.sync.dma_start(
                    out=xT[:, :nch, :],
                    in_=nloc_T[:, w0 * 128:(w0 + nch) * 128]
                        .rearrange("p (c n) -> p c n", n=128))
                ps = ppool.tile([128, RIFF, 128], F32, space="PSUM",
                                tag="proj")
                for c in range(nch):
                    nc.tensor.matmul(ps[:, c, :], lhsT=xT[:, c, :],
                                     rhs=wr_t[:], start=True, stop=True,
                                     skip_group_check=True)
                nc.vector.tensor_tensor(
                    r_sb[:, w0:w0 + nch, :], ps[:, :nch, :],
                    wrbr_t[:].unsqueeze(1).to_broadcast([128, nch, 128]),
                    op=mybir.AluOpType.add)
                w0 += nch

            # ------------- phase 2: one window per iteration ----------------
            tabm = {"lo": tab_lo, "hi": tab_hi}
            for w in range(NWIN):
                sidx_t = stpool.tile([128, wine // 16], I16, tag="sidx")
                rloc_t = stpool.tile([128, cpw], F16, tag="rloc")
                ohT = gpool.tile([128, cpw, 128], F16, tag="ohT")
                nc.sync.dma_start(out=sidx_t[:], in_=sidx_in[w])
                nc.sync.dma_start(out=rloc_t[:], in_=rloc_in[w])
                nc.sync.dma_start(
                    out=ohT[:],
                    in_=ohT_in[w].rearrange("p (c n) -> p c n", n=128))
                s_t = gpool.tile([128, cpw, 128], F16, tag="s_t")
                for tab, cs, n in segs:
                    nc.gpsimd.dma_gather(
                        out_ap=s_t[:, cs:cs + n, :], in_ap=tabm[tab][:],
                        idxs_ap=sidx_t[:, cs * 8:(cs + n) * 8],
                        num_idxs=n * CHUNK, num_idxs_reg=n * CHUNK,
                        elem_size=F)

                oh = wpool.tile([128, cpw, 128], F16, tag="oh")
                nc.vector.tensor_tensor(
                    oh[:],
                    rloc_t[:].unsqueeze(2).to_broadcast([128, cpw, 128]),
                    iota_t[:].unsqueeze(1).to_broadcast([128, cpw, 128]),
                    op=mybir.AluOpType.is_equal)

                # r one-hot gather on PE + x = s + r
                x = wpool.tile([128, cpw, 128], F16, tag="x")
                for r0, rn in riffs:
                    r_ps = psR.tile([128, RIFF, 128], F32, space="PSUM",
                                    tag="r_ps")
                    for j in range(rn):
                        nc.tensor.matmul(r_ps[:, j, :],
                                         lhsT=ohT[:, r0 + j, :],
                                         rhs=r_sb[:, w, :],
                                         start=True, stop=True,
                                         skip_group_check=True)
                    nc.vector.tensor_tensor(
                        x[:, r0:r0 + rn, :], s_t[:, r0:r0 + rn, :],
                        r_ps[:, :rn, :], op=mybir.AluOpType.add)

                # mish: t = tanh(softplus(x)) via u=e^x, w=(u+1)^2,
                # t = 1 - 2/(w+1);  f32 chain (exp can't overflow f32,
                # and reciprocal_approx_fast is f32-only)
                uw = wpool.tile([128, cpw, 128], F32, tag="uw")
                nc.scalar.activation(uw[:], x[:],
                                     mybir.ActivationFunctionType.Exp)
                nc.scalar.activation(uw[:], uw[:],
                                     mybir.ActivationFunctionType.Square,
                                     bias=1.0)
                nc.scalar.activation(uw[:], uw[:],
                                     mybir.ActivationFunctionType.Copy,
                                     bias=1.0)
                rr = wpool.tile([128, cpw, 128], F32, tag="rr")
                nc.vector.reciprocal_approx_fast(rr[:], uw[:])
                t = wpool.tile([128, cpw, 128], F16, tag="t")
                nc.scalar.activation(t[:], rr[:],
                                     mybir.ActivationFunctionType.Copy,
                                     scale=c_m2[:], bias=1.0)
                hm = wpool.tile([128, cpw, 128], F16, tag="hm")
                nc.vector.tensor_tensor(hm[:], x[:], t[:],
                                        op=mybir.AluOpType.mult)
                nc.vector.tensor_tensor(
                    hm[:], hm[:],
                    attn_t[:].unsqueeze(1).to_broadcast([128, cpw, 128]),
                    op=mybir.AluOpType.mult)
                lgt = wpool.tile([128, cpw, H], F16, tag="lgt")
                nc.vector.tensor_reduce(
                    out=lgt[:].unsqueeze(3),
                    in_=hm[:].rearrange("p c (h d) -> p c h d", d=D),
                    op=mybir.AluOpType.add, axis=mybir.AxisListType.X)
                msgp = wpool.tile([128, cpw, F + H], F16, tag="msgp")
                nc.scalar.activation(msgp[:, :, F:F + H], lgt[:],
                                     mybir.ActivationFunctionType.Exp,
                                     bias=b_exp[:])
                nc.vector.tensor_tensor(
                    msgp[:, :, 0:F].rearrange("p c (h d) -> p c h d", d=D),
                    s_t[:].rearrange("p c (h d) -> p c h d", d=D),
                    msgp[:, :, F:F + H].unsqueeze(3)
                        .to_broadcast([128, cpw, H, D]),
                    op=mybir.AluOpType.mult)

                agg_ps = psA.tile([128, F + H], F32, space="PSUM",
                                  tag="agg")
                for c in range(cpw):
                    nc.tensor.matmul(agg_ps[:], lhsT=oh[:, c, :],
                                     rhs=msgp[:, c, :], start=(c == 0),
                                     stop=(c == cpw - 1),
                                     skip_group_check=True)
                nc.scalar.copy(acc[:, w, :], agg_ps[:])

                # normalize + store finished batch of windows
                if w % NB == NB - 1 or w == NWIN - 1:
                    wb = w - w % NB
                    nb = w - wb + 1
                    den = acc[:, wb:w + 1, F:F + H]
                    nc.vector.tensor_scalar_add(den, den, 1e-30)
                    rcp = wpool.tile([128, NB, H], F32, tag="rcp")
                    nc.vector.reciprocal(rcp[:, :nb, :], den)
                    outb = wpool.tile([128, NB, F], F32, tag="outb")
                    nc.vector.tensor_tensor(
                        outb[:, :nb, :].rearrange("p w (h d) -> p w h d", d=D),
                        acc[:, wb:w + 1, 0:F]
                            .rearrange("p w (h d) -> p w h d", d=D),
                        rcp[:, :nb, :].unsqueeze(3)
                            .to_broadcast([128, nb, H, D]),
                        op=mybir.AluOpType.mult)
                    nc.sync.dma_start(
                        out=out_d[wb * 128:(w + 1) * 128, :]
                            .rearrange("(w p) f -> p w f", p=128),
                        in_=outb[:, :nb, :])

    nc.compile()
    return nc


def _balance(deg, nbins, cap):
    """Serpentine-deal nodes (sorted by degree desc) into nbins bins.

    Returns bin id per node.  Each bin gets ceil/floor(n/nbins) nodes,
    and degree sums are near-equal."""
    n = len(deg)
    order = np.argsort(-deg, kind="stable")
    bins = np.empty(n, np.int64)
    pattern = np.concatenate([np.arange(nbins), np.arange(nbins)[::-1]])
    reps = (n + 2 * nbins - 1) // (2 * nbins)
    seq = np.tile(pattern, reps)[:n]
    bins[order] = seq
    assert np.bincount(bins, minlength=nbins).max() <= cap
    return bins


def _balance2d(dlo, dhi, nbins, cap):
    """Greedy 2-D balance: nodes (desc by total degree) go to the bin
    minimizing the max of normalized (lo, hi) loads, node-count capped."""
    n = len(dlo)
    mlo = max(dlo.sum() / nbins, 1.0)
    mhi = max(dhi.sum() / nbins, 1.0)
    order = np.argsort(-(dlo + dhi), kind="stable")
    lo_w = np.zeros(nbins)
    hi_w = np.zeros(nbins)
    cnt = np.zeros(nbins, np.int64)
    bins = np.empty(n, np.int64)
    for i in order:
        cost = np.maximum((lo_w + dlo[i]) / mlo, (hi_w + dhi[i]) / mhi)
        cost[cnt >= cap] = np.inf
        b = int(np.argmin(cost))
        bins[i] = b
        lo_w[b] += dlo[i]
        hi_w[b] += dhi[i]
        cnt[b] += 1
    return bins


def _prep(senders, receivers):
    """Host-side layout: permutation, per-core index arrays."""
    half = (senders >= SPLIT).astype(np.int64)
    deg_lo = np.bincount(receivers[half == 0], minlength=N_NODES)
    deg_hi = np.bincount(receivers[half == 1], minlength=N_NODES)
    deg = deg_lo + deg_hi

    core_of = _balance(deg, NCORE, NPC)
    win_of = np.empty(N_NODES, np.int64)
    slot_of = np.empty(N_NODES, np.int64)
    for c in range(NCORE):
        idx = np.nonzero(core_of == c)[0]
        w = _balance2d(deg_lo[idx], deg_hi[idx], NWIN, WIN)
        win_of[idx] = w
        # slot = position within window
        for ww in range(NWIN):
            ii = idx[w == ww]
            slot_of[ii] = np.arange(len(ii))

    # per (core, window, half) counts -> uniform chunk structure
    e_core = core_of[receivers]
    e_win = win_of[receivers]
    key = (e_core * NWIN + e_win) * 2 + half
    counts = np.bincount(key, minlength=NCORE * NWIN * 2).reshape(-1, 2)
    L_ch = max(1, int(np.ceil(counts[:, 0].max() / CHUNK)))
    H_ch = max(1, int(np.ceil(counts[:, 1].max() / CHUNK)))
    return core_of, win_of, slot_of, half, L_ch, H_ch


def _core_arrays(senders, receivers, core_of, win_of, slot_of, half,
                 core, L_ch, H_ch):
    cpw = L_ch + H_ch
    wine = cpw * CHUNK
    mask = core_of[receivers] == core
    s = senders[mask].astype(np.int64)
    hf = half[mask]
    w = win_of[receivers[mask]]
    sl = slot_of[receivers[mask]]

    sidx_val = np.zeros(NWIN * wine, np.int64)
    rloc_val = np.full(NWIN * wine, 999.0, np.float32)
    nre = np.zeros((NWIN, 2), np.int64)
    order = np.lexsort((hf, w))
    s, hf, w, sl = s[order], hf[order], w[order], sl[order]
    for ww in range(NWIN):
        for h in (0, 1):
            g = (w == ww) & (hf == h)
            n = int(g.sum())
            nre[ww, h] = n
            if n == 0:
                continue
            base = ww * wine + (L_ch * CHUNK if h else 0)
            cap = (H_ch if h else L_ch) * CHUNK
            assert n <= cap
            sidx_val[base:base + n] = s[g] - (SPLIT if h else 0)
            rloc_val[base:base + n] = sl[g]

    v = sidx_val.reshape(NWIN, wine // 16, 16).astype(np.int16)
    sidx = np.tile(np.transpose(v, (0, 2, 1)), (1, 8, 1)).copy()
    rl = rloc_val.reshape(NWIN, cpw, CHUNK)          # [w, c, e]
    rloc = rl.transpose(0, 2, 1).astype(np.float16).copy()
    ar = np.arange(128, dtype=np.float32)
    # ohT[w, n, c, e] = (rl[w,c,e] == n)
    ohT = (rl[:, :, None, :] == ar[:, None]).astype(np.float16)
    ohT = ohT.transpose(0, 2, 1, 3).reshape(NWIN, 128, wine).copy()

    # per-seg runtime gather counts (>=16, multiple of 16)
    cnt = np.zeros((1, NWIN * 8), np.int32)
    segs = []
    pos = 0
    for nch_total, h in ((L_ch, 0), (H_ch, 1)):
        left = nch_total
        cs0 = pos
        while left > 0:
            n = min(GCAP, left)
            segs.append((h, (pos - cs0) * CHUNK, n))  # (half, off-in-half, n)
            pos += n
            left -= n
    for ww in range(NWIN):
        for si, (h, off, n) in enumerate(segs):
            c = int(np.clip(nre[ww, h] - off, 0, n * CHUNK))
            c = max(16, ((c + 15) // 16) * 16)
            cnt[0, ww * 8 + si] = c

    # permuted local node slice, transposed: column w*128+slot
    nidx = np.nonzero(core_of == core)[0]
    cols = win_of[nidx] * WIN + slot_of[nidx]
    return sidx, rloc, ohT, cnt, nidx, cols


def kernel(nodes, senders, receivers, Ws_k, Ws_b, Wr_k, Wr_b, attn_k, attn_b):
    nodes = np.asarray(nodes, np.float32)
    senders = np.asarray(senders, np.int32)
    receivers = np.asarray(receivers, np.int32)
    assert nodes.shape == (N_NODES, F) and senders.shape == (N_EDGES,)

    core_of, win_of, slot_of, half, L_ch, H_ch = _prep(senders, receivers)
    exp_bias = float(np.asarray(attn_b).ravel()[0]) - 2.0

    ck = (L_ch, H_ch, exp_bias)
    if ck not in _prog_cache:
        _prog_cache[ck] = _build_program(*ck)
    nc = _prog_cache[ck]

    nodes_T = np.zeros((F, NP_PAD), np.float16)
    nodes_T[:, :N_NODES] = nodes.T.astype(np.float16)
    ws_mat = np.asarray(Ws_k, np.float32).reshape(F, F).astype(np.float16)
    wr_mat = np.asarray(Wr_k, np.float32).reshape(F, F).astype(np.float16)
    wsb_rep = np.broadcast_to(np.asarray(Ws_b, np.float32).reshape(1, F),
                              (128, F)).astype(np.float16).copy()
    wrb_rep = np.broadcast_to(np.asarray(Wr_b, np.float32).reshape(1, F),
                              (128, F)).astype(np.float16).copy()
    a_flat = np.tile(np.asarray(attn_k, np.float32).ravel(), H)
    attn_rep = np.broadcast_to(a_flat, (128, 128)).astype(np.float16).copy()
    iota = np.broadcast_to(np.arange(128, dtype=np.float16),
                           (128, 128)).copy()

    in_maps = []
    metas = []
    for c in range(NCORE):
        sidx, rloc, ohT, cnt, nidx, cols = _core_arrays(
            senders, receivers, core_of, win_of, slot_of, half, c, L_ch, H_ch)
        nloc_T = np.zeros((F, NSLOT), np.float16)
        nloc_T[:, cols] = nodes[nidx].T.astype(np.float16)
        metas.append((nidx, cols))
        in_maps.append({
            "nodes_T": nodes_T, "nloc_T": nloc_T,
            "ws_mat": ws_mat, "wr_mat": wr_mat,
            "wsb_rep": wsb_rep, "wrb_rep": wrb_rep,
            "iota": iota, "attn_rep": attn_rep,
            "sidx": sidx, "rloc": rloc, "ohT": ohT,
        })

    trace = bool(int(os.environ.get("GAT_TRACE", "0")))
    res = bass_utils.run_bass_kernel_spmd(nc, in_maps,
                                          core_ids=list(range(NCORE)),
                                          trace=trace)
    if trace:
        kernel.last_profile = res
    out = np.empty((N_NODES, F), np.float32)
    for c in range(NCORE):
        nidx, cols = metas[c]
        out[nidx] = np.asarray(res.results[c]["out_d"])[cols]
    return out
